# revision 1
# baseline (speedup 1.0000x reference)
# Causal self-attention kernel for 8 Trainium2 NeuronCores.
#
# Problem (hardcoded): B=2, S=2048, D=1024, H=16 heads of dk=64.
#   q,k,v = x @ W.T + b (torch Linear), per-head causal softmax attention,
#   out[b,s,:] = concat_h(attn_h @ v_h). No output projection.
#
# Sharding: 8 cores = 2 batches x 4 head-groups. Core c handles batch c//4
# and heads [4*(c%4), 4*(c%4)+4) => output channels [256*(c%4), +256).
# No cross-device communication.
#
# Per-core design (engine-balance driven):
#   - ACT is the critical engine: softmax exp runs only there (~1 elem/cyc/
#     lane + ~352cyc fixed cost per ACTIVATE). Scores are built in big
#     [128, <=1024] PSUM segments so exp runs as few, wide calls as possible.
#   - All matmuls use float32r (full PE rate at moving-dim>=256, ~2e-4 rel
#     error). Weights pre-transposed/augmented on CPU (parameter packing).
#   - x is PE-transposed to xT (d on partitions); qT/kT [e,s] and v [s,e+1]
#     projections; v carries a ones column so the PV matmul also produces
#     softmax denominators (row 64 of the accumulator).
#   - Attention per (head, sq-half): key-block j outer; scoresT[sk,sq] on PE,
#     additive -1e9 causal mask on the diagonal block, one exp per segment,
#     PV accumulates outT_aug[65, 1024] in PSUM across j.
#   - Tail: copy accumulator to SBUF, PE-transpose back incl. sums row,
#     reciprocal + per-partition scale into the output staging tile.
#   - Emission interleaving: only head 0/1's first-half prerequisites are
#     emitted up front; the rest of the projections are drip-fed into the
#     attention phase ("filler"), filling PE while ACT grinds through exp.

import numpy as np

B, S, D, H = 2, 2048, 1024, 16
DK = D // H            # 64
NCORES = 8
HPC = 4                # heads per core
E = HPC * DK           # 256 output channels per core
EA = HPC * (DK + 1)    # 260 augmented v width (ones col per head)
P = 128
NSB = S // P           # 16 s-blocks
NDC = D // P           # 8 d-chunks
HALF = 1024

_cache = {}


def _build_module():
    import concourse.bacc as bacc
    import concourse.mybir as mybir
    import concourse.tile as tile

    f32 = mybir.dt.float32
    f32r = mybir.dt.float32r
    Exp = mybir.ActivationFunctionType.Exp
    Copy = mybir.ActivationFunctionType.Copy

    nc = bacc.Bacc("TRN2", target_bir_lowering=False, debug=False)

    x_d = nc.dram_tensor("x", [S, D], f32r, kind="ExternalInput")
    wq_d = nc.dram_tensor("wq_t", [D, E], f32r, kind="ExternalInput")
    wk_d = nc.dram_tensor("wk_t", [D, E], f32r, kind="ExternalInput")
    wv_d = nc.dram_tensor("wv_t", [D, EA], f32r, kind="ExternalInput")
    bq_d = nc.dram_tensor("bq", [1, E], f32, kind="ExternalInput")
    bk_d = nc.dram_tensor("bk", [1, E], f32, kind="ExternalInput")
    bv_d = nc.dram_tensor("bv", [1, EA], f32r, kind="ExternalInput")
    mask_d = nc.dram_tensor("mask", [P, P], f32r, kind="ExternalInput")
    ident_d = nc.dram_tensor("ident", [P, P], f32, kind="ExternalInput")
    ones_d = nc.dram_tensor("ones", [1, P], f32r, kind="ExternalInput")
    out_d = nc.dram_tensor("out", [S, E], f32, kind="ExternalOutput")

    with tile.TileContext(nc) as tc:
        with (
            tc.tile_pool(name="consts", bufs=1) as consts,
            tc.tile_pool(name="qkv", bufs=1) as qkv,
            tc.tile_pool(name="outst", bufs=1) as outst,
            tc.tile_pool(name="xin", bufs=4) as xin,
            tc.tile_pool(name="xt", bufs=1) as xtp,
            tc.tile_pool(name="pp", bufs=2, space="PSUM") as pp,
            tc.tile_pool(name="psc", bufs=2, space="PSUM") as pscp,
            tc.tile_pool(name="pacc", bufs=1, space="PSUM") as paccp,
            tc.tile_pool(name="attn", bufs=3) as attnp,
            tc.tile_pool(name="otn", bufs=2) as otnp,
        ):
            # ---- constants ----
            wq_sb = consts.tile([P, NDC, E], f32r, tag="wq")
            wk_sb = consts.tile([P, NDC, E], f32r, tag="wk")
            wv_sb = consts.tile([P, NDC, EA], f32r, tag="wv")
            bqc_sb = consts.tile([P, 2], f32, tag="bqc")
            bkc_sb = consts.tile([P, 2], f32, tag="bkc")
            bv_sb = consts.tile([1, EA], f32r, tag="bv")
            mask_sb = consts.tile([P, P], f32r, tag="mask")
            identr_sb = consts.tile([P, P], f32r, tag="identr")
            ident_sb = consts.tile([P, P], f32, tag="ident")
            ones_sb = consts.tile([1, P], f32r, tag="ones")

            x_tiles = {}

            def emit_x_dma(sb):
                x_tile = xin.tile([P, D], f32r, tag="x")
                nc.sync.dma_start(out=x_tile, in_=x_d[sb * P:(sb + 1) * P, :])
                x_tiles[sb] = x_tile

            for _sb in range(8):
                emit_x_dma(_sb)

            nc.sync.dma_start(out=wq_sb, in_=wq_d[:].rearrange("(c p) e -> p c e", p=P))
            nc.sync.dma_start(out=wk_sb, in_=wk_d[:].rearrange("(c p) e -> p c e", p=P))
            nc.sync.dma_start(out=wv_sb, in_=wv_d[:].rearrange("(c p) e -> p c e", p=P))
            nc.sync.dma_start(out=bqc_sb, in_=bq_d[:].rearrange("o (c p) -> p (o c)", p=P))
            nc.sync.dma_start(out=bkc_sb, in_=bk_d[:].rearrange("o (c p) -> p (o c)", p=P))
            nc.sync.dma_start(out=bv_sb, in_=bv_d[:])
            nc.sync.dma_start(out=mask_sb, in_=mask_d[:])
            nc.sync.dma_start(out=ident_sb, in_=ident_d[:])
            nc.sync.dma_start(out=identr_sb, in_=ident_d[:].bitcast(f32r))
            nc.sync.dma_start(out=ones_sb, in_=ones_d[:])

            qT = qkv.tile([P, 2, S], f32r, tag="qT")
            kT = qkv.tile([P, 2, S], f32r, tag="kT")
            v_sb = qkv.tile([P, NSB, EA], f32r, tag="v")
            out_sb = outst.tile([P, NSB, E], f32, tag="out")
            # xT: [d%128, s-block, d-chunk, 128] so transpose copies are
            # contiguous 512-wide (d-chunk groups of 4)
            xT = xtp.tile([P, NSB, NDC, P], f32r, tag="xT")

            nalt = [0]  # alternator for copy engine balance
            ACT_COPIES = False  # exp owns ACT; keep its queue clear

            def copy_balanced(dst, src, act_ok=True):
                if ACT_COPIES and act_ok and nalt[0] % 2 == 0:
                    nc.scalar.copy(dst, src)
                else:
                    nc.vector.tensor_copy(dst, src)
                nalt[0] += 1

            def emit_xt(sb, dcg, act_ok=True):
                # transpose 4 d-chunks of x block sb into one psum tile
                if dcg == 0 and sb not in x_tiles:
                    emit_x_dma(sb)
                ptile = pp.tile([P, 512], f32r, tag="pp")
                for k in range(4):
                    dc = dcg * 4 + k
                    nc.tensor.transpose(
                        ptile[:, k * P:(k + 1) * P],
                        x_tiles[sb][:, dc * P:(dc + 1) * P],
                        identr_sb,
                    )
                copy_balanced(xT[:, sb, dcg * 4:(dcg + 1) * 4, :], ptile, act_ok)

            def qk_rhs(dc, lo, w):
                # xT view for d-chunk dc, s columns [lo, lo+w) (128-aligned)
                sb0 = lo // P
                return xT[:, sb0:sb0 + w // P, dc, :]

            def emit_qk_proj(which, eb, sc, act_ok=True):
                w_sb = wq_sb if which == 0 else wk_sb
                bc = bqc_sb if which == 0 else bkc_sb
                dst = qT if which == 0 else kT
                ps = pp.tile([P, 512], f32, tag="pp")
                for dc in range(NDC):
                    nc.tensor.matmul(
                        ps,
                        lhsT=w_sb[:, dc, eb * P:(eb + 1) * P],
                        rhs=qk_rhs(dc, sc * 512, 512),
                        start=(dc == 0),
                        stop=(dc == NDC - 1),
                    )
                dst_ap = dst[:, eb, sc * 512:(sc + 1) * 512]
                nc.vector.tensor_scalar_add(dst_ap, ps, bc[:, eb:eb + 1])
                nalt[0] += 1

            def emit_v_proj(sb, act_ok=True):
                ps = pp.tile([P, 512], f32, tag="pp")
                pv = ps[:, :EA]
                for dc in range(NDC):
                    nc.tensor.matmul(
                        pv,
                        lhsT=xT[:, sb, dc, :],
                        rhs=wv_sb[:, dc, :],
                        start=(dc == 0),
                        stop=False,
                    )
                nc.tensor.matmul(
                    pv,
                    lhsT=ones_sb[0:1, :],
                    rhs=bv_sb[0:1, :],
                    start=False,
                    stop=True,
                )
                copy_balanced(v_sb[:, sb, :], pv, act_ok)

            # ---- phase A, sliced so attention starts early ----
            def phase_a_slice1():
                for sb in range(8):
                    emit_xt(sb, 0)
                    emit_xt(sb, 1)
                emit_qk_proj(0, 0, 0)
                emit_qk_proj(0, 0, 1)
                emit_qk_proj(1, 0, 0)
                for sb in (0, 1):
                    emit_v_proj(sb)

            filler = [lambda: emit_qk_proj(1, 0, 1, act_ok=False)]
            for sb in range(2, 8):
                filler.append(lambda sb=sb: emit_v_proj(sb, act_ok=False))
            for sc in (0, 1):
                for which in (0, 1):
                    filler.append(lambda which=which, sc=sc: emit_qk_proj(
                        which, 1, sc, act_ok=False))
            for sb in range(8, NSB):
                filler.append(lambda sb=sb: emit_xt(sb, 0, act_ok=False))
                filler.append(lambda sb=sb: emit_xt(sb, 1, act_ok=False))
            for sb in range(8, NSB):
                filler.append(lambda sb=sb: emit_v_proj(sb, act_ok=False))
            for sc in (2, 3):
                for which in (0, 1):
                    filler.append(lambda which=which, sc=sc: emit_qk_proj(
                        which, 0, sc, act_ok=False))
            for sc in (2, 3):
                for which in (0, 1):
                    filler.append(lambda which=which, sc=sc: emit_qk_proj(
                        which, 1, sc, act_ok=False))

            def drain(n):
                for _ in range(n):
                    if filler:
                        filler.pop(0)()

            # ---- attention ----
            def attn_head_half(h, half, jhook=None, per_block_dma=False):
                po = 64 * (h % 2)
                eb = h // 2
                kT_h = kT[po:po + DK, eb, :]
                qT_h = qT[po:po + DK, eb, :]
                lo = half * HALF
                hi = lo + HALF
                pacc = paccp.tile([65, HALF], f32, tag="pacc")

                def emit_pv(j, at):
                    # PV pieces: absolute 512-aligned within [lo, hi)
                    sb0 = max(j * P, lo)
                    lhsT_v = v_sb[:, j, h * 65:(h + 1) * 65]
                    m = sb0
                    while m < hi:
                        w = min(512 - (m - lo) % 512, hi - m)
                        bank = (m - lo) // 512
                        j_last = min((lo + 512 * (bank + 1)) // P - 1, hi // P - 1)
                        nc.tensor.matmul(
                            pacc[:, m - lo:m - lo + w],
                            lhsT=lhsT_v,
                            rhs=at[:, m - sb0:m - sb0 + w],
                            start=(j == 0),
                            stop=(j == j_last),
                        )
                        m += w

                pending = None  # (j, at) whose PV is deferred one iteration
                for j in range(hi // P):
                    ko = j * P
                    sb0 = max(ko, lo)
                    segw = hi - sb0
                    ps = pscp.tile([P, HALF], f32, tag="sc")
                    lhsT_k = kT_h[:, ko:ko + P]
                    m = 0
                    while m < segw:
                        w = min(512, segw - m)
                        if w < 256 and sb0 + m + 256 <= hi:
                            w = 256  # pad narrow f32r pieces to full rate
                        nc.tensor.matmul(
                            ps[:, m:m + w],
                            lhsT=lhsT_k,
                            rhs=qT_h[:, sb0 + m:sb0 + m + w],
                            start=True,
                            stop=True,
                        )
                        m += w
                    at = attnp.tile([P, HALF], f32r, tag="at")
                    nc.scalar.activation(
                        out=at[:, :segw], in_=ps[:, :segw], func=Exp, scale=0.125
                    )
                    if ko >= lo:
                        nc.gpsimd.tensor_mul(at[:, 0:P], at[:, 0:P], mask_sb)
                    if jhook is not None:
                        jhook(j)
                    if pending is not None:
                        emit_pv(*pending)
                    pending = (j, at)
                emit_pv(*pending)

                # tail: normalize + transpose back + stage
                otn = otnp.tile([65, HALF], f32, tag="otn")
                nc.vector.tensor_copy(otn, pacc)
                for il in range(HALF // P):
                    i = half * 8 + il
                    pot = pp.tile([P, 65], f32, tag="pp")
                    nc.tensor.transpose(
                        pot, otn[:, il * P:(il + 1) * P], ident_sb[0:65, 0:65]
                    )
                    linv = otnp.tile([P, 1], f32, tag="linv")
                    nc.vector.reciprocal(linv, pot[:, DK:DK + 1])
                    nc.vector.tensor_scalar_mul(
                        out_sb[:, i, h * DK:(h + 1) * DK], pot[:, 0:DK], linv
                    )
                    if per_block_dma:
                        nc.sync.dma_start(
                            out=out_d[i * P:(i + 1) * P, :], in_=out_sb[:, i, :]
                        )

            phase_a_slice1()
            hook = lambda j: drain(2)
            # prereq positions: (0,1)/(1,1) need everything through eb0-sc23
            # (34 units); (2,*) need eb1 (+8). 16 j-slots before (0,1) at
            # 2/j = 32; force-drain the remainder at each boundary.
            attn_head_half(0, 0, jhook=hook)
            attn_head_half(1, 0, jhook=hook)
            attn_head_half(2, 0, jhook=hook)
            attn_head_half(3, 0, jhook=hook)
            for i in range(8):
                nc.sync.dma_start(
                    out=out_d[i * P:(i + 1) * P, :], in_=out_sb[:, i, :]
                )
            drain(len(filler))
            attn_head_half(0, 1)
            attn_head_half(1, 1)
            attn_head_half(2, 1)
            attn_head_half(3, 1, per_block_dma=True)

    nc.compile()
    return nc


def _prep_core_inputs(inputs, c):
    x = np.ascontiguousarray(np.asarray(inputs["x"], dtype=np.float32))
    b, hg = c // HPC, c % HPC
    e0 = hg * E

    wq = np.asarray(inputs["Wq"], dtype=np.float32)
    wk = np.asarray(inputs["Wk"], dtype=np.float32)
    wv = np.asarray(inputs["Wv"], dtype=np.float32)
    bq = np.asarray(inputs["bq"], dtype=np.float32)
    bk = np.asarray(inputs["bk"], dtype=np.float32)
    bv = np.asarray(inputs["bv"], dtype=np.float32)

    wq_t = np.ascontiguousarray(wq[e0:e0 + E, :].T)          # [D, E]
    wk_t = np.ascontiguousarray(wk[e0:e0 + E, :].T)
    wv_t = np.zeros((D, EA), dtype=np.float32)
    bv_a = np.zeros((1, EA), dtype=np.float32)
    for lh in range(HPC):
        cols = slice(lh * 65, lh * 65 + DK)
        rows = slice(e0 + lh * DK, e0 + lh * DK + DK)
        wv_t[:, cols] = wv[rows, :].T
        bv_a[0, cols] = bv[rows]
        bv_a[0, lh * 65 + DK] = 1.0                          # ones column

    mask = np.where(
        np.arange(P)[None, :] >= np.arange(P)[:, None], 1.0, 0.0
    ).astype(np.float32)

    return {
        "x": np.ascontiguousarray(x[b]),
        "wq_t": wq_t,
        "wk_t": wk_t,
        "wv_t": wv_t,
        "bq": np.ascontiguousarray(bq[e0:e0 + E])[None, :],
        "bk": np.ascontiguousarray(bk[e0:e0 + E])[None, :],
        "bv": bv_a,
        "mask": mask,
        "ident": np.eye(P, dtype=np.float32),
        "ones": np.ones((1, P), dtype=np.float32),
    }


def kernel(**inputs):
    from concourse.bass_utils import run_bass_kernel_spmd

    if "nc" not in _cache:
        _cache["nc"] = _build_module()
    nc = _cache["nc"]

    in_maps = [_prep_core_inputs(inputs, c) for c in range(NCORES)]
    res = run_bass_kernel_spmd(nc, in_maps, core_ids=list(range(NCORES)))

    out = np.empty((B, S, D), dtype=np.float32)
    for c in range(NCORES):
        b, hg = c // HPC, c % HPC
        out[b, :, hg * E:(hg + 1) * E] = res.results[c]["out"]
    return out



# revision 18
# speedup vs baseline: 2.3430x; 2.3430x over previous
# Causal self-attention kernel for 8 Trainium2 NeuronCores.
#
# Problem (hardcoded): B=2, S=2048, D=1024, H=16 heads of dk=64.
#   q,k,v = x @ W.T + b (torch Linear), per-head causal softmax attention,
#   out[b,s,:] = concat_h(attn_h @ v_h). No output projection.
#
# Sharding: 8 cores = 2 batches x 4 head-groups. Core c handles batch c//4
# and heads [4*(c%4), 4*(c%4)+4) => output channels [256*(c%4), +256).
# No cross-device communication.
#
# Per-core design (engine-balance driven):
#   - All matmuls use float32r (full PE rate at moving-dim>=256, ~2e-4 rel
#     error). Weights pre-transposed/augmented on CPU (parameter packing).
#   - x is PE-transposed to xT (d on partitions); qT/kT [e,s] and v [s,e+1]
#     projections; v carries a ones column per head so the PV matmul also
#     produces softmax denominators (row 64 of the accumulator).
#   - Attention runs per (head-PAIR, 512-wide sq-chunk). The two heads of a
#     pair live on disjoint PE partition ranges (0:64 / 64:128), so their
#     score matmuls target different row-groups and pack concurrently in
#     the PE array (tile_position auto-derived from base partitions).
#     Scores for the pair land interleaved in one [128, 2, 512] PSUM tile
#     (one bank per head), so a single ACT exp covers both heads per
#     key-block j. Diagonal-block causal masking is a multiplicative 0/1
#     mask on gpsimd after exp.
#   - PV accumulates out_aug [65, 2, 512] (64 ch + denominator row per
#     head) in PSUM across j; tail: copy to SBUF, PE-transpose back per
#     128-query block, reciprocal + per-partition scale into out staging.
#   - Emission interleaving: only the first chunk's prerequisites are
#     emitted up front; the rest of the projections are drip-fed into the
#     attention phase ("filler"), filling PE while ACT grinds through exp.
#   - reps: emits the whole kernel body `reps` times in one NEFF (fresh
#     tile pools each rep). Used by the test harness to measure steady-
#     state per-iteration device time via a slope over reps, which cancels
#     per-exec dispatch overhead. kernel() itself uses reps=1.

import numpy as np

B, S, D, H = 2, 2048, 1024, 16
DK = D // H            # 64
NCORES = 8
HPC = 4                # heads per core
E = HPC * DK           # 256 output channels per core
EA = HPC * (DK + 1)    # 260 augmented v width (ones col per head)
P = 128
NSB = S // P           # 16 s-blocks
NDC = D // P           # 8 d-chunks
CW = 512               # attention sq-chunk width
NCH = S // CW          # 4 chunks

_cache = {}


def _build_module(reps=1):
    from contextlib import ExitStack

    import concourse.bacc as bacc
    import concourse.mybir as mybir
    import concourse.tile as tile

    f32 = mybir.dt.float32
    f32r = mybir.dt.float32r
    Exp = mybir.ActivationFunctionType.Exp

    nc = bacc.Bacc("TRN2", target_bir_lowering=False, debug=False)

    x_d = nc.dram_tensor("x", [S, D], f32r, kind="ExternalInput")
    wq_d = nc.dram_tensor("wq_t", [D, E], f32r, kind="ExternalInput")
    wk_d = nc.dram_tensor("wk_t", [D, E], f32r, kind="ExternalInput")
    wv_d = nc.dram_tensor("wv_t", [D, EA], f32r, kind="ExternalInput")
    bq_d = nc.dram_tensor("bq", [1, E], f32, kind="ExternalInput")
    bk_d = nc.dram_tensor("bk", [1, E], f32, kind="ExternalInput")
    bv_d = nc.dram_tensor("bv", [1, EA], f32r, kind="ExternalInput")
    mask_d = nc.dram_tensor("mask", [P, P], f32r, kind="ExternalInput")
    ident_d = nc.dram_tensor("ident", [P, P], f32, kind="ExternalInput")
    ones_d = nc.dram_tensor("ones", [1, P], f32r, kind="ExternalInput")
    out_d = nc.dram_tensor("out", [S, E], f32, kind="ExternalOutput")

    with tile.TileContext(nc) as tc:
        for rep in range(reps):
            with ExitStack() as ctx:
                _build_rep(nc, tc, ctx, tile, mybir, rep,
                           x_d, wq_d, wk_d, wv_d, bq_d, bk_d, bv_d,
                           mask_d, ident_d, ones_d, out_d)

    nc.compile()
    return nc


def _build_rep(nc, tc, ctx, tile, mybir, rep,
               x_d, wq_d, wk_d, wv_d, bq_d, bk_d, bv_d,
               mask_d, ident_d, ones_d, out_d):
    f32 = mybir.dt.float32
    f32r = mybir.dt.float32r
    Exp = mybir.ActivationFunctionType.Exp

    consts = ctx.enter_context(tc.tile_pool(name=f"consts{rep}", bufs=1))
    qkv = ctx.enter_context(tc.tile_pool(name=f"qkv{rep}", bufs=1))
    outst = ctx.enter_context(tc.tile_pool(name=f"outst{rep}", bufs=1))
    xin = ctx.enter_context(tc.tile_pool(name=f"xin{rep}", bufs=4))
    xtp = ctx.enter_context(tc.tile_pool(name=f"xt{rep}", bufs=1))
    pp = ctx.enter_context(tc.tile_pool(name=f"pp{rep}", bufs=2, space="PSUM"))
    pscp = ctx.enter_context(
        tc.tile_pool(name=f"psc{rep}", bufs=2, space="PSUM"))
    paccp = ctx.enter_context(
        tc.tile_pool(name=f"pacc{rep}", bufs=1, space="PSUM"))
    attnp = ctx.enter_context(tc.tile_pool(name=f"attn{rep}", bufs=3))
    otnp = ctx.enter_context(tc.tile_pool(name=f"otn{rep}", bufs=2))

    # ---- constants ----
    wq_sb = consts.tile([P, NDC, E], f32r, tag="wq")
    wk_sb = consts.tile([P, NDC, E], f32r, tag="wk")
    wv_sb = consts.tile([P, NDC, EA], f32r, tag="wv")
    bqc_sb = consts.tile([P, 2], f32, tag="bqc")
    bkc_sb = consts.tile([P, 2], f32, tag="bkc")
    bv_sb = consts.tile([1, EA], f32r, tag="bv")
    mask_sb = consts.tile([P, P], f32r, tag="mask")
    identr_sb = consts.tile([P, P], f32r, tag="identr")
    ident_sb = consts.tile([P, P], f32, tag="ident")
    ones_sb = consts.tile([1, P], f32r, tag="ones")

    x_tiles = {}

    def emit_x_dma(sb):
        x_tile = xin.tile([P, D], f32r, tag="x")
        nc.sync.dma_start(out=x_tile, in_=x_d[sb * P:(sb + 1) * P, :])
        x_tiles[sb] = x_tile

    # Two HWDGE queues: x tiles (+ identity, needed first by the
    # transposes) stream on the SP queue while weights/biases stream in
    # parallel on the Activation queue (idle until the first exp anyway).
    nc.sync.dma_start(out=identr_sb, in_=ident_d[:].bitcast(f32r))
    nc.scalar.dma_start(out=ident_sb, in_=ident_d[:])
    nc.scalar.dma_start(out=wq_sb, in_=wq_d[:].rearrange("(c p) e -> p c e", p=P))
    nc.scalar.dma_start(out=wk_sb, in_=wk_d[:].rearrange("(c p) e -> p c e", p=P))
    nc.scalar.dma_start(out=bqc_sb, in_=bq_d[:].rearrange("o (c p) -> p (o c)", p=P))
    nc.scalar.dma_start(out=bkc_sb, in_=bk_d[:].rearrange("o (c p) -> p (o c)", p=P))
    nc.scalar.dma_start(out=mask_sb, in_=mask_d[:])
    nc.scalar.dma_start(out=ones_sb, in_=ones_d[:])
    for _sb in range(4):
        emit_x_dma(_sb)
    nc.sync.dma_start(out=wv_sb, in_=wv_d[:].rearrange("(c p) e -> p c e", p=P))
    nc.sync.dma_start(out=bv_sb, in_=bv_d[:])

    qT = qkv.tile([P, 2, S], f32r, tag="qT")
    kT = qkv.tile([P, 2, S], f32r, tag="kT")
    v_sb = qkv.tile([P, NSB, EA], f32r, tag="v")
    out_sb = outst.tile([P, NSB, E], f32, tag="out")
    # xT: [d%128, s-block, d-chunk, 128] so transpose copies are
    # contiguous 512-wide (d-chunk groups of 4)
    xT = xtp.tile([P, NSB, NDC, P], f32r, tag="xT")

    nalt = [0]

    def emit_xt(sb, dcg, act_ok=False):
        # transpose 4 d-chunks of x block sb into one psum tile
        if dcg == 0 and sb not in x_tiles:
            emit_x_dma(sb)
        ptile = pp.tile([P, 512], f32r, tag="pp")
        for k in range(4):
            dc = dcg * 4 + k
            nc.tensor.transpose(
                ptile[:, k * P:(k + 1) * P],
                x_tiles[sb][:, dc * P:(dc + 1) * P],
                identr_sb,
            )
        dst = xT[:, sb, dcg * 4:(dcg + 1) * 4, :]
        nalt[0] += 1
        if act_ok and nalt[0] % 2 == 0:
            nc.scalar.copy(dst, ptile)
        else:
            nc.vector.tensor_copy(dst, ptile)

    def emit_qk_proj(which, eb, sc):
        w_sb = wq_sb if which == 0 else wk_sb
        bc = bqc_sb if which == 0 else bkc_sb
        dst = qT if which == 0 else kT
        ps = pp.tile([P, 512], f32, tag="pp")
        sb0 = sc * 512 // P
        for dc in range(NDC):
            nc.tensor.matmul(
                ps,
                lhsT=w_sb[:, dc, eb * P:(eb + 1) * P],
                rhs=xT[:, sb0:sb0 + 4, dc, :],
                start=(dc == 0),
                stop=(dc == NDC - 1),
            )
        dst_ap = dst[:, eb, sc * 512:(sc + 1) * 512]
        nc.vector.tensor_scalar_add(dst_ap, ps, bc[:, eb:eb + 1])

    def emit_v_proj(sb):
        ps = pp.tile([P, 512], f32, tag="pp")
        pv = ps[:, :EA]
        for dc in range(NDC):
            nc.tensor.matmul(
                pv,
                lhsT=xT[:, sb, dc, :],
                rhs=wv_sb[:, dc, :],
                start=(dc == 0),
                stop=False,
            )
        nc.tensor.matmul(
            pv,
            lhsT=ones_sb[0:1, :],
            rhs=bv_sb[0:1, :],
            start=False,
            stop=True,
        )
        nc.vector.tensor_copy(v_sb[:, sb, :], pv)

    # ---- phase A slice: chunk 0 / pair 0 prerequisites only ----
    def phase_a_slice1():
        for sb in range(4):
            emit_xt(sb, 0)
            emit_xt(sb, 1)
        emit_qk_proj(0, 0, 0)
        emit_qk_proj(1, 0, 0)
        for sb in range(4):
            emit_v_proj(sb)

    # Remaining projection work, drip-fed into the attention phase.
    # prep[(pair, c)] holds the filler units for the NEXT unit's
    # prerequisites; each unit consumes its own list evenly across its
    # j-steps (interleaving PE-filler work between attention matmuls so
    # neither PE nor DVE sees a contiguous burst).
    def fq(which, eb, sc):
        return lambda: emit_qk_proj(which, eb, sc)

    def fxt(sb, dcg):
        return lambda: emit_xt(sb, dcg, act_ok=True)

    def fv(sb):
        return lambda: emit_v_proj(sb)

    def xtv_group(s0):
        # interleave transposes with v-projections so PE/DVE alternate
        units = []
        for sb in range(s0, s0 + 4):
            units.append(fxt(sb, 0))
            units.append(fxt(sb, 1))
            units.append(fv(sb))
        return units

    prep = {
        (0, 0): [fq(0, 1, 0), fq(1, 1, 0)] + xtv_group(4)[:3],
        (1, 0): xtv_group(4)[3:] + [fq(0, 0, 1), fq(1, 0, 1)],
        (0, 1): [fq(0, 1, 1), fq(1, 1, 1)] + xtv_group(8)[:3],
        (1, 1): xtv_group(8)[3:] + [fq(0, 0, 2), fq(1, 0, 2)],
        (0, 2): [fq(0, 1, 2), fq(1, 1, 2)] + xtv_group(12)[:3],
        (1, 2): xtv_group(12)[3:] + [fq(0, 0, 3), fq(1, 0, 3)],
        (0, 3): [fq(0, 1, 3), fq(1, 1, 3)],
        (1, 3): [],
    }

    # ---- attention: one (head-pair, sq-chunk) unit ----
    def attn_pair_chunk(pair, c):
        eb = pair
        lo, hi = c * CW, (c + 1) * CW
        nj = hi // P
        my_prep = prep[(pair, c)]
        consumed = [0]

        def drain_paced(jl):
            tgt = min(-(-len(my_prep) * (jl + 1) // nj), len(my_prep))
            while consumed[0] < tgt:
                my_prep[consumed[0]]()
                consumed[0] += 1

        pacc = paccp.tile([65, 2, CW], f32, tag="pacc")

        def emit_pv(j, at, off, w):
            for h01 in (0, 1):
                h = 2 * pair + h01
                nc.tensor.matmul(
                    pacc[:, h01, off:off + w],
                    lhsT=v_sb[:, j, h * 65:(h + 1) * 65],
                    rhs=at[:, h01, off:off + w],
                    start=(j == 0),
                    stop=(j == nj - 1),
                )

        pending = None  # (j, at, off, w): PV deferred one iteration
        for j in range(nj):
            ko = j * P
            sb0 = max(ko, lo)
            off = sb0 - lo
            w = hi - sb0
            ps = pscp.tile([P, 2, CW], f32, tag="sc")
            for h01 in (0, 1):
                po = DK * h01
                # the pair's two matmuls hit disjoint PE row-groups
                # (partitions 0:64 / 64:128) -> they pack concurrently
                nc.tensor.matmul(
                    ps[:, h01, off:off + w],
                    lhsT=kT[po:po + DK, eb, ko:ko + P],
                    rhs=qT[po:po + DK, eb, sb0:sb0 + w],
                    start=True,
                    stop=True,
                )
            at = attnp.tile([P, 2, CW], f32r, tag="at")
            nc.scalar.activation(
                out=at[:, :, off:off + w], in_=ps[:, :, off:off + w],
                func=Exp, scale=0.125,
            )
            if ko >= lo:
                for h01 in (0, 1):
                    nc.gpsimd.tensor_mul(
                        at[:, h01, off:off + P], at[:, h01, off:off + P],
                        mask_sb,
                    )
            drain_paced(j)
            if pending is not None:
                emit_pv(*pending)
            pending = (j, at, off, w)
        emit_pv(*pending)

        # tail: normalize + transpose back + stage (per-block copies so
        # the first transpose starts before the whole pacc is drained)
        otn = otnp.tile([65, 2, CW], f32, tag="otn")
        for il in range(CW // P):
            nc.vector.tensor_copy(
                otn[:, :, il * P:(il + 1) * P], pacc[:, :, il * P:(il + 1) * P]
            )
            i = c * (CW // P) + il
            for h01 in (0, 1):
                h = 2 * pair + h01
                pot = pp.tile([P, 65], f32, tag="pp")
                nc.tensor.transpose(
                    pot, otn[:, h01, il * P:(il + 1) * P],
                    ident_sb[0:65, 0:65],
                )
                linv = otnp.tile([P, 1], f32, tag="linv")
                nc.vector.reciprocal(linv, pot[:, DK:DK + 1])
                nc.vector.tensor_scalar_mul(
                    out_sb[:, i, h * DK:(h + 1) * DK], pot[:, 0:DK], linv
                )
            if pair == 1:
                nc.sync.dma_start(
                    out=out_d[i * P:(i + 1) * P, :], in_=out_sb[:, i, :]
                )

    phase_a_slice1()
    for c in range(NCH):
        for pair in (0, 1):
            attn_pair_chunk(pair, c)


def _prep_core_inputs(inputs, c):
    x = np.ascontiguousarray(np.asarray(inputs["x"], dtype=np.float32))
    b, hg = c // HPC, c % HPC
    e0 = hg * E

    wq = np.asarray(inputs["Wq"], dtype=np.float32)
    wk = np.asarray(inputs["Wk"], dtype=np.float32)
    wv = np.asarray(inputs["Wv"], dtype=np.float32)
    bq = np.asarray(inputs["bq"], dtype=np.float32)
    bk = np.asarray(inputs["bk"], dtype=np.float32)
    bv = np.asarray(inputs["bv"], dtype=np.float32)

    wq_t = np.ascontiguousarray(wq[e0:e0 + E, :].T)          # [D, E]
    wk_t = np.ascontiguousarray(wk[e0:e0 + E, :].T)
    wv_t = np.zeros((D, EA), dtype=np.float32)
    bv_a = np.zeros((1, EA), dtype=np.float32)
    for lh in range(HPC):
        cols = slice(lh * 65, lh * 65 + DK)
        rows = slice(e0 + lh * DK, e0 + lh * DK + DK)
        wv_t[:, cols] = wv[rows, :].T
        bv_a[0, cols] = bv[rows]
        bv_a[0, lh * 65 + DK] = 1.0                          # ones column
    mask = np.where(
        np.arange(P)[None, :] >= np.arange(P)[:, None], 1.0, 0.0
    ).astype(np.float32)

    return {
        "x": np.ascontiguousarray(x[b]),
        "wq_t": wq_t,
        "wk_t": wk_t,
        "wv_t": wv_t,
        "bq": np.ascontiguousarray(bq[e0:e0 + E])[None, :],
        "bk": np.ascontiguousarray(bk[e0:e0 + E])[None, :],
        "bv": bv_a,
        "mask": mask,
        "ident": np.eye(P, dtype=np.float32),
        "ones": np.ones((1, P), dtype=np.float32),
    }


def kernel(**inputs):
    from concourse.bass_utils import run_bass_kernel_spmd

    if "nc" not in _cache:
        _cache["nc"] = _build_module()
    nc = _cache["nc"]

    in_maps = [_prep_core_inputs(inputs, c) for c in range(NCORES)]
    res = run_bass_kernel_spmd(nc, in_maps, core_ids=list(range(NCORES)))

    out = np.empty((B, S, D), dtype=np.float32)
    for c in range(NCORES):
        b, hg = c // HPC, c % HPC
        out[b, :, hg * E:(hg + 1) * E] = res.results[c]["out"]
    return out


# revision 28
# speedup vs baseline: 5.2476x; 2.2397x over previous
# Causal self-attention kernel for 8 Trainium2 NeuronCores.
#
# Problem (hardcoded): B=2, S=2048, D=1024, H=16 heads of dk=64.
#   q,k,v = x @ W.T + b (torch Linear), per-head causal softmax attention,
#   out[b,s,:] = concat_h(attn_h @ v_h). No output projection.
#
# Sharding: 8 cores = 2 batches x 4 head-groups. Core c handles batch c//4
# and heads [4*(c%4), 4*(c%4)+4) => output channels [256*(c%4), +256).
# No cross-device communication.
#
# Per-core design (engine-balance driven):
#   - All matmuls use float32r (full PE rate at moving-dim>=256, ~2e-4 rel
#     error). Weights pre-transposed/augmented on CPU (parameter packing).
#   - x is PE-transposed to xT (d on partitions); qT/kT [e,s] and v [s,e+1]
#     projections; v carries a ones column per head so the PV matmul also
#     produces softmax denominators (row 64 of the accumulator).
#   - Attention runs per (head-PAIR, 512-wide sq-chunk). The two heads of a
#     pair live on disjoint PE partition ranges (0:64 / 64:128), so their
#     score matmuls target different row-groups and pack concurrently in
#     the PE array (tile_position auto-derived from base partitions).
#     Scores for the pair land interleaved in one [128, 2, 512] PSUM tile
#     (one bank per head), so a single ACT exp covers both heads per
#     key-block j. Diagonal-block causal masking is a multiplicative 0/1
#     mask on gpsimd after exp.
#   - PV accumulates out_aug [65, 2, 512] (64 ch + denominator row per
#     head) in PSUM across j; tail: copy to SBUF, PE-transpose back per
#     128-query block, reciprocal + per-partition scale into out staging.
#   - Emission interleaving: only the first chunk's prerequisites are
#     emitted up front; the rest of the projections are drip-fed into the
#     attention phase ("filler"), filling PE while ACT grinds through exp.
#   - reps: emits the whole kernel body `reps` times in one NEFF (fresh
#     tile pools each rep). Used by the test harness to measure steady-
#     state per-iteration device time via a slope over reps, which cancels
#     per-exec dispatch overhead. kernel() itself uses reps=1.

import numpy as np

B, S, D, H = 2, 2048, 1024, 16
DK = D // H            # 64
NCORES = 8
HPC = 4                # heads per core
E = HPC * DK           # 256 output channels per core
EA = HPC * (DK + 1)    # 260 augmented v width (ones col per head)
P = 128
NSB = S // P           # 16 s-blocks
NDC = D // P           # 8 d-chunks
CW = 512               # attention sq-chunk width
NCH = S // CW          # 4 chunks

_cache = {}


def _build_module(reps=1):
    from contextlib import ExitStack

    import concourse.bacc as bacc
    import concourse.mybir as mybir
    import concourse.tile as tile

    f32 = mybir.dt.float32
    f32r = mybir.dt.float32r
    Exp = mybir.ActivationFunctionType.Exp

    nc = bacc.Bacc("TRN2", target_bir_lowering=False, debug=False)

    x_d = nc.dram_tensor("x", [S, D], f32r, kind="ExternalInput")
    wq_d = nc.dram_tensor("wq_t", [D, E], f32r, kind="ExternalInput")
    wk_d = nc.dram_tensor("wk_t", [D, E], f32r, kind="ExternalInput")
    wv_d = nc.dram_tensor("wv_t", [D, EA], f32r, kind="ExternalInput")
    bq_d = nc.dram_tensor("bq", [1, E], f32, kind="ExternalInput")
    bk_d = nc.dram_tensor("bk", [1, E], f32, kind="ExternalInput")
    bv_d = nc.dram_tensor("bv", [P, EA], f32r, kind="ExternalInput")
    mask_d = nc.dram_tensor("mask", [P, P], f32r, kind="ExternalInput")
    ident_d = nc.dram_tensor("ident", [P, P], f32, kind="ExternalInput")
    out_d = nc.dram_tensor("out", [S, E], f32, kind="ExternalOutput")

    with tile.TileContext(nc) as tc:
        for rep in range(reps):
            with ExitStack() as ctx:
                _build_rep(nc, tc, ctx, tile, mybir, rep,
                           x_d, wq_d, wk_d, wv_d, bq_d, bk_d, bv_d,
                           mask_d, ident_d, out_d)

    nc.compile()
    return nc


def _build_rep(nc, tc, ctx, tile, mybir, rep,
               x_d, wq_d, wk_d, wv_d, bq_d, bk_d, bv_d,
               mask_d, ident_d, out_d):
    f32 = mybir.dt.float32
    f32r = mybir.dt.float32r
    Exp = mybir.ActivationFunctionType.Exp

    consts = ctx.enter_context(tc.tile_pool(name=f"consts{rep}", bufs=1))
    qkv = ctx.enter_context(tc.tile_pool(name=f"qkv{rep}", bufs=1))
    outst = ctx.enter_context(tc.tile_pool(name=f"outst{rep}", bufs=1))
    xin = ctx.enter_context(tc.tile_pool(name=f"xin{rep}", bufs=4))
    xtp = ctx.enter_context(tc.tile_pool(name=f"xt{rep}", bufs=1))
    pp = ctx.enter_context(tc.tile_pool(name=f"pp{rep}", bufs=2, space="PSUM"))
    pscp = ctx.enter_context(
        tc.tile_pool(name=f"psc{rep}", bufs=2, space="PSUM"))
    paccp = ctx.enter_context(
        tc.tile_pool(name=f"pacc{rep}", bufs=1, space="PSUM"))
    attnp = ctx.enter_context(tc.tile_pool(name=f"attn{rep}", bufs=3))
    otnp = ctx.enter_context(tc.tile_pool(name=f"otn{rep}", bufs=2))

    # ---- constants ----
    wq_sb = consts.tile([P, NDC, E], f32r, tag="wq")
    wk_sb = consts.tile([P, NDC, E], f32r, tag="wk")
    wv_sb = consts.tile([P, NDC, EA], f32r, tag="wv")
    bqc_sb = consts.tile([P, 2], f32, tag="bqc")
    bkc_sb = consts.tile([P, 2], f32, tag="bkc")
    bv_sb = consts.tile([P, EA], f32r, tag="bv")
    mask_sb = consts.tile([P, P], f32r, tag="mask")
    identr_sb = consts.tile([P, P], f32r, tag="identr")
    ident_sb = consts.tile([P, P], f32, tag="ident")

    x_tiles = {}

    def emit_x_dma(sb):
        # two half-width DMAs: the dcg-0 transposes only need cols 0:512,
        # so they start as soon as the first half lands
        x_tile = xin.tile([P, D], f32r, tag="x")
        nc.sync.dma_start(
            out=x_tile[:, 0:D // 2], in_=x_d[sb * P:(sb + 1) * P, 0:D // 2]
        )
        nc.sync.dma_start(
            out=x_tile[:, D // 2:D], in_=x_d[sb * P:(sb + 1) * P, D // 2:D]
        )
        x_tiles[sb] = x_tile

    # Two HWDGE queues: x tiles (+ identity, needed first by the
    # transposes) stream on the SP queue while weights/biases stream in
    # parallel on the Activation queue (idle until the first exp anyway).
    nc.sync.dma_start(out=identr_sb, in_=ident_d[:].bitcast(f32r))
    nc.scalar.dma_start(out=ident_sb, in_=ident_d[:])
    nc.scalar.dma_start(out=wq_sb, in_=wq_d[:].rearrange("(c p) e -> p c e", p=P))
    nc.scalar.dma_start(out=wk_sb, in_=wk_d[:].rearrange("(c p) e -> p c e", p=P))
    nc.scalar.dma_start(out=bqc_sb, in_=bq_d[:].rearrange("o (c p) -> p (o c)", p=P))
    nc.scalar.dma_start(out=bkc_sb, in_=bk_d[:].rearrange("o (c p) -> p (o c)", p=P))
    nc.scalar.dma_start(out=mask_sb, in_=mask_d[:])
    for _sb in range(4):
        emit_x_dma(_sb)
    nc.sync.dma_start(out=wv_sb, in_=wv_d[:].rearrange("(c p) e -> p c e", p=P))
    nc.sync.dma_start(out=bv_sb, in_=bv_d[:])

    qT = qkv.tile([P, 2, S], f32r, tag="qT")
    kT = qkv.tile([P, 2, S], f32r, tag="kT")
    v_sb = qkv.tile([P, NSB, EA], f32r, tag="v")
    out_sb = outst.tile([P, NSB, E], f32, tag="out")
    # xT: [d%128, s-block, d-chunk, 128] so transpose copies are
    # contiguous 512-wide (d-chunk groups of 4)
    xT = xtp.tile([P, NSB, NDC, P], f32r, tag="xT")

    nalt = [0]

    def emit_xt(sb, dcg, act_ok=False):
        # transpose 4 d-chunks of x block sb into one psum tile
        if dcg == 0 and sb not in x_tiles:
            emit_x_dma(sb)
        ptile = pp.tile([P, 512], f32r, tag="pp")
        for k in range(4):
            dc = dcg * 4 + k
            nc.tensor.transpose(
                ptile[:, k * P:(k + 1) * P],
                x_tiles[sb][:, dc * P:(dc + 1) * P],
                identr_sb,
            )
        dst = xT[:, sb, dcg * 4:(dcg + 1) * 4, :]
        nalt[0] += 1
        if act_ok and nalt[0] % 2 == 0:
            nc.scalar.copy(dst, ptile)
        else:
            nc.vector.tensor_copy(dst, ptile)

    def emit_qk_proj(which, eb, sc):
        w_sb = wq_sb if which == 0 else wk_sb
        bc = bqc_sb if which == 0 else bkc_sb
        dst = qT if which == 0 else kT
        ps = pp.tile([P, 512], f32, tag="pp")
        sb0 = sc * 512 // P
        for dc in range(NDC):
            nc.tensor.matmul(
                ps,
                lhsT=w_sb[:, dc, eb * P:(eb + 1) * P],
                rhs=xT[:, sb0:sb0 + 4, dc, :],
                start=(dc == 0),
                stop=(dc == NDC - 1),
            )
        dst_ap = dst[:, eb, sc * 512:(sc + 1) * 512]
        nc.vector.tensor_scalar_add(dst_ap, ps, bc[:, eb:eb + 1])

    def emit_v_proj(sb):
        ps = pp.tile([P, 512], f32, tag="pp")
        pv = ps[:, :EA]
        for dc in range(NDC):
            nc.tensor.matmul(
                pv,
                lhsT=xT[:, sb, dc, :],
                rhs=wv_sb[:, dc, :],
                start=(dc == 0),
                stop=(dc == NDC - 1),
            )
        # bias (incl. the 1.0 of each head's ones column) fused into the
        # PSUM->SBUF move as a partition-broadcast add
        nc.vector.tensor_add(v_sb[:, sb, :], pv, bv_sb)

    # ---- phase A slice: chunk 0 / pair 0 prerequisites only ----
    def phase_a_slice1():
        for sb in range(4):
            emit_xt(sb, 0)
            emit_xt(sb, 1)
        emit_qk_proj(0, 0, 0)
        emit_qk_proj(1, 0, 0)
        for sb in range(4):
            emit_v_proj(sb)

    # Remaining projection work, drip-fed into the attention phase.
    # prep[(pair, c)] holds the filler units for the NEXT unit's
    # prerequisites; each unit consumes its own list evenly across its
    # j-steps (interleaving PE-filler work between attention matmuls so
    # neither PE nor DVE sees a contiguous burst).
    def fq(which, eb, sc):
        return lambda: emit_qk_proj(which, eb, sc)

    def fxt(sb, dcg):
        return lambda: emit_xt(sb, dcg, act_ok=True)

    def fv(sb):
        return lambda: emit_v_proj(sb)

    def xtv_group(s0):
        # interleave transposes with v-projections so PE/DVE alternate
        units = []
        for sb in range(s0, s0 + 4):
            units.append(fxt(sb, 0))
            units.append(fxt(sb, 1))
            units.append(fv(sb))
        return units

    prep = {
        (0, 0): [fq(0, 1, 0), fq(1, 1, 0)] + xtv_group(4)[:3],
        (1, 0): xtv_group(4)[3:] + [fq(0, 0, 1), fq(1, 0, 1)],
        (0, 1): [fq(0, 1, 1), fq(1, 1, 1)] + xtv_group(8)[:3],
        (1, 1): xtv_group(8)[3:] + [fq(0, 0, 2), fq(1, 0, 2)],
        (0, 2): [fq(0, 1, 2), fq(1, 1, 2)] + xtv_group(12)[:3],
        (1, 2): xtv_group(12)[3:] + [fq(0, 0, 3), fq(1, 0, 3)],
        (0, 3): [fq(0, 1, 3), fq(1, 1, 3)],
        (1, 3): [],
    }

    # ---- attention: one (head-pair, sq-chunk) unit ----
    def attn_pair_chunk(pair, c):
        eb = pair
        lo, hi = c * CW, (c + 1) * CW
        nj = hi // P
        my_prep = prep[(pair, c)]
        consumed = [0]

        def drain_paced(jl):
            tgt = min(-(-len(my_prep) * (jl + 1) // nj), len(my_prep))
            while consumed[0] < tgt:
                my_prep[consumed[0]]()
                consumed[0] += 1

        pacc = paccp.tile([65, 2, CW], f32, tag="pacc")

        def emit_pv(j, at, off, w):
            # pad 128-wide diagonal pieces to 256 (full f32r rate needs
            # moving-dim >= 256); the extra at columns are zeroed, so the
            # accumulated contribution there is 0
            o0 = off - 128 if w == 128 else off
            for h01 in (0, 1):
                h = 2 * pair + h01
                nc.tensor.matmul(
                    pacc[:, h01, o0:off + w],
                    lhsT=v_sb[:, j, h * 65:(h + 1) * 65],
                    rhs=at[:, h01, o0:off + w],
                    start=(j == 0),
                    stop=(j == nj - 1),
                )

        # tail, drained per 128-query block: block il receives its last PV
        # contribution at j = 4c + il (its diagonal), so its normalize/
        # transpose/stage can run while later j's still accumulate.
        otn = otnp.tile([65, 2, CW], f32, tag="otn")

        def emit_block_tail(il):
            nc.vector.tensor_copy(
                otn[:, :, il * P:(il + 1) * P], pacc[:, :, il * P:(il + 1) * P]
            )
            i = c * (CW // P) + il
            for h01 in (0, 1):
                h = 2 * pair + h01
                pot = pp.tile([P, 65], f32, tag="pp")
                nc.tensor.transpose(
                    pot, otn[:, h01, il * P:(il + 1) * P],
                    ident_sb[0:65, 0:65],
                )
                linv = otnp.tile([P, 1], f32, tag="linv")
                nc.vector.reciprocal(linv, pot[:, DK:DK + 1])
                nc.vector.tensor_scalar_mul(
                    out_sb[:, i, h * DK:(h + 1) * DK], pot[:, 0:DK], linv
                )
            if pair == 1:
                nc.sync.dma_start(
                    out=out_d[i * P:(i + 1) * P, :], in_=out_sb[:, i, :]
                )

        pending = None  # (j, at, off, w): PV deferred one iteration
        for j in range(nj):
            ko = j * P
            sb0 = max(ko, lo)
            off = sb0 - lo
            w = hi - sb0
            ps = pscp.tile([P, 2, CW], f32, tag="sc")
            s0 = sb0 - 128 if w == 128 else sb0  # 256-wide diagonal pad
            for h01 in (0, 1):
                po = DK * h01
                # the pair's two matmuls hit disjoint PE row-groups
                # (partitions 0:64 / 64:128) -> they pack concurrently
                nc.tensor.matmul(
                    ps[:, h01, s0 - lo:off + w],
                    lhsT=kT[po:po + DK, eb, ko:ko + P],
                    rhs=qT[po:po + DK, eb, s0:sb0 + w],
                    start=True,
                    stop=True,
                )
            at = attnp.tile([P, 2, CW], f32r, tag="at")
            nc.scalar.activation(
                out=at[:, :, off:off + w], in_=ps[:, :, off:off + w],
                func=Exp, scale=0.125,
            )
            if ko >= lo:
                for h01 in (0, 1):
                    nc.gpsimd.tensor_mul(
                        at[:, h01, off:off + P], at[:, h01, off:off + P],
                        mask_sb,
                    )
            if w == 128:
                # zero the pad columns feeding the widened PV piece
                # (f32 view: ISA memset rejects the f32r dtype)
                nc.gpsimd.memset(at[:, :, off - 128:off].bitcast(f32), 0.0)
            drain_paced(j)
            if pending is not None:
                emit_pv(*pending)
                pj = pending[0]
                if pj >= nj - 4:
                    emit_block_tail(pj - (nj - 4))
            pending = (j, at, off, w)
        emit_pv(*pending)
        emit_block_tail(3)

    phase_a_slice1()
    for c in range(NCH):
        for pair in (0, 1):
            attn_pair_chunk(pair, c)


def _prep_core_inputs(inputs, c):
    x = np.ascontiguousarray(np.asarray(inputs["x"], dtype=np.float32))
    b, hg = c // HPC, c % HPC
    e0 = hg * E

    wq = np.asarray(inputs["Wq"], dtype=np.float32)
    wk = np.asarray(inputs["Wk"], dtype=np.float32)
    wv = np.asarray(inputs["Wv"], dtype=np.float32)
    bq = np.asarray(inputs["bq"], dtype=np.float32)
    bk = np.asarray(inputs["bk"], dtype=np.float32)
    bv = np.asarray(inputs["bv"], dtype=np.float32)

    wq_t = np.ascontiguousarray(wq[e0:e0 + E, :].T)          # [D, E]
    wk_t = np.ascontiguousarray(wk[e0:e0 + E, :].T)
    wv_t = np.zeros((D, EA), dtype=np.float32)
    bv_a = np.zeros((1, EA), dtype=np.float32)
    for lh in range(HPC):
        cols = slice(lh * 65, lh * 65 + DK)
        rows = slice(e0 + lh * DK, e0 + lh * DK + DK)
        wv_t[:, cols] = wv[rows, :].T
        bv_a[0, cols] = bv[rows]
        bv_a[0, lh * 65 + DK] = 1.0                          # ones column
    mask = np.where(
        np.arange(P)[None, :] >= np.arange(P)[:, None], 1.0, 0.0
    ).astype(np.float32)

    return {
        "x": np.ascontiguousarray(x[b]),
        "wq_t": wq_t,
        "wk_t": wk_t,
        "wv_t": wv_t,
        "bq": np.ascontiguousarray(bq[e0:e0 + E])[None, :],
        "bk": np.ascontiguousarray(bk[e0:e0 + E])[None, :],
        "bv": np.ascontiguousarray(np.tile(bv_a, (P, 1))),
        "mask": mask,
        "ident": np.eye(P, dtype=np.float32),
    }


def kernel(**inputs):
    from concourse.bass_utils import run_bass_kernel_spmd

    if "nc" not in _cache:
        _cache["nc"] = _build_module()
    nc = _cache["nc"]

    in_maps = [_prep_core_inputs(inputs, c) for c in range(NCORES)]
    res = run_bass_kernel_spmd(nc, in_maps, core_ids=list(range(NCORES)))

    out = np.empty((B, S, D), dtype=np.float32)
    for c in range(NCORES):
        b, hg = c // HPC, c % HPC
        out[b, :, hg * E:(hg + 1) * E] = res.results[c]["out"]
    return out
